# revision 23
# baseline (speedup 1.0000x reference)
"""AWQ W4 grouped-dequant matmul on 8 Trainium2 cores.

y = (x / s) @ (w_q * scales).reshape(OUT, IN).T + bias

Column-parallel sharding: each core owns OUT/8 = 1376 output channels
(padded to 1408 = 11*128), x is replicated. Per core the kernel computes
y_shard^T [1408, 2048] = W'[1408, 4096] @ x_bf16[4096, 2048] where the
smoothing division is folded into the weights: W' = (w_q * scales) / s.

Schedule (v2): the PE starts ~2us in and chases the W DMA stream.
  - DMA priority order: small tables first, then per k-tile [w8, scales,
    x(chunk 0) piece] interleaved, then x(1), then y-stores/x(2)/x(3)
    phased behind s_pe progress. All on the SP HWDGE queue in program
    order.
  - Wave 1: 8 PSUM banks accumulate groups (tt=0, ot=0..7) kc-major, so
    each arriving W k-tile feeds 8 matmuls (8 x 213ns); the wave is
    paced by the ~2.2us/k-tile DMA stream.
  - W ships as fp8e4m3 (w_q in [-7,7] is exact in e4m3), halving the W
    stream to 5.8MB; scales ship replicated x128 in bf16 (layout move
    only). DVE dequants each k-tile with one full-width
    scalar_tensor_tensor: wd_bf16 = (w8 * (1/s)_k) * sc, from a rotating
    fp8 staging slot into the resident bf16 W.
  - PSUM eviction runs on the otherwise-idle Scalar engine as
    activation(Identity, bias=bias[o]) straight into a 4-slot rotating
    bf16 buffer; plain DMAs stream it out.
  - gpsimd ISA ops (partition_broadcast etc.) do not compile in this
    toolchain ("ISA wrong length" in walrus codegen), hence the
    host-replicated scales.

The toolchain permits AT MOST ONE semaphore wait per instruction. All
waits are standalone engine instructions (EVENT_SEMAPHORE); every DMA or
compute op carries only its completion increment. In-order engine streams
make earlier waits cover later instructions transitively, so steady-state
matmuls carry no waits at all.

Host side does only layout/dtype moves: transpose, pad, shard, bf16
casts (w_q ints are exact in bf16).
"""

import os
from contextlib import ExitStack

import numpy as np

# ---- problem constants (hardcoded per contract) ----
OUT, N_GROUPS, GROUP = 11008, 32, 128
IN = N_GROUPS * GROUP  # 4096
TOKENS = 2048
N_CORES = 8
P = 128
O_SHARD = OUT // N_CORES  # 1376
O_PAD = 1408  # 11 * 128
OT = O_PAD // P  # 11 o-tiles
KT = IN // P  # 32 k-tiles (== quant groups, GROUP == P)
TCH = 512  # tokens per chunk == PSUM bank free size (f32)
NT = TOKENS // TCH  # 4 chunks
NG = NT * OT  # 44 psum groups
NB = 8  # psum banks; wave 1 accumulates 8 groups at once
WAVE = NB  # groups in wave 1 (tt=0, ot=0..7)
XB = 2  # x chunk buffers
SCB = 4  # broadcast-scales staging slots
NYS = 4  # y eviction slots

LAST = {}  # exec_time_ns etc. for the local test harness

_NC_CACHE = {}


def _mm_end(n):
    """Global matmul count after group n's last matmul retires."""
    # wave 1 is kc-major interleaved over groups 0..7; groups 8..43 are
    # sequential, 32 matmuls each.
    return 249 + n if n < WAVE else 8 * KT + KT * (n - WAVE + 1)


def _build_nc():
    import concourse.bass as bass
    from concourse import mybir

    f32 = mybir.dt.float32
    bf16 = mybir.dt.bfloat16
    fp8 = mybir.dt.float8e4

    nc = bass.Bass()
    xT = nc.declare_dram_parameter("xT", [IN, TOKENS], bf16, isOutput=False)
    w8T = nc.declare_dram_parameter("w8T", [IN, O_PAD], fp8, isOutput=False)
    sc_repl = nc.declare_dram_parameter(
        "sc_repl", [N_GROUPS, P, O_PAD], bf16, isOutput=False
    )
    s_cols = nc.declare_dram_parameter("s_cols", [P, KT], f32, isOutput=False)
    bias_cols = nc.declare_dram_parameter("bias_cols", [P, OT], f32, isOutput=False)
    yT = nc.declare_dram_parameter("yT", [O_PAD, TOKENS], bf16, isOutput=True)

    with ExitStack() as ctx:
        w_all = ctx.enter_context(nc.sbuf_tensor("w_all", [P, KT * O_PAD], bf16))
        xn_all = ctx.enter_context(nc.sbuf_tensor("xn_all", [P, XB * KT * TCH], bf16))
        y_sl = ctx.enter_context(nc.sbuf_tensor("y_sl", [P, NYS * TCH], bf16))
        scb = ctx.enter_context(nc.sbuf_tensor("scb", [P, SCB * O_PAD], bf16))
        w8s = ctx.enter_context(nc.sbuf_tensor("w8s", [P, SCB * O_PAD], fp8))
        s_sb = ctx.enter_context(nc.sbuf_tensor("s_sb", [P, KT], f32))
        inv_s = ctx.enter_context(nc.sbuf_tensor("inv_s", [P, KT], f32))
        bias_sb = ctx.enter_context(nc.sbuf_tensor("bias_sb", [P, OT], f32))
        ps = [
            ctx.enter_context(nc.psum_tensor(f"ps{i}", [P, TCH], f32))
            for i in range(NB)
        ]
        # DMA completions can reorder across the 16 engines, so every wait
        # must target a semaphore whose increments are sequenced: per-item
        # sems for the streamed W/x(0) tiles, terminal-only waits for bulk
        # chunks, per-slot sems (self-sequencing via the evict/store cycle)
        # for y, and single-producer engine sems elsewhere.
        s_tbl = ctx.enter_context(nc.semaphore("s_tbl"))
        s_wd = [ctx.enter_context(nc.semaphore(f"s_wd{k}")) for k in range(KT)]
        s_x0 = [ctx.enter_context(nc.semaphore(f"s_x0_{k}")) for k in range(KT)]
        s_xc = [ctx.enter_context(nc.semaphore(f"s_xc{t}")) for t in range(1, NT)]
        s_dq = ctx.enter_context(nc.semaphore("s_dq"))
        s_pe = ctx.enter_context(nc.semaphore("s_pe"))
        s_act = ctx.enter_context(nc.semaphore("s_act"))
        s_ys = [ctx.enter_context(nc.semaphore(f"s_ys{j}")) for j in range(NYS)]
        block = ctx.enter_context(nc.Block())

        def wd(kc):
            return w_all[:, kc * O_PAD : (kc + 1) * O_PAD]

        def xreg(tt, kc):
            o = ((tt % XB) * KT + kc) * TCH
            return xn_all[:, o : o + TCH]

        def yslot(n):
            o = (n % NYS) * TCH
            return y_sl[:, o : o + TCH]

        def sslot(kc):
            o = (kc % SCB) * O_PAD
            return scb[:, o : o + O_PAD]

        def w8slot(kc):
            o = (kc % SCB) * O_PAD
            return w8s[:, o : o + O_PAD]

        @block.sync
        def _(sync):
            # tables (terminal wait s_tbl >= 32; completions may reorder)
            sync.dma_start(out=s_sb[:, :], in_=s_cols[:, :]).then_inc(s_tbl, 16)
            sync.dma_start(out=bias_sb[:, :], in_=bias_cols[:, :]).then_inc(s_tbl, 16)
            # wave-1 stream: per kc [w8 tile, scales tile, x(0) piece].
            # w8/scales land in rotating staging slots; the slot-reuse WAR
            # wait (dequant of kc-SCB retired) is pre-satisfied in practice
            # because the stt chases ~1 tile behind the DMA stream while
            # the wait is ~SCB tiles back. s_dq is single-producer (DVE),
            # so any intermediate wait value is race-free.
            for kc in range(KT):
                if kc >= SCB:
                    sync.wait_ge(s_dq, kc - SCB + 2)
                sync.dma_start(
                    out=w8slot(kc), in_=w8T[kc * P : (kc + 1) * P, :]
                ).then_inc(s_wd[kc], 16)
                sync.dma_start(out=sslot(kc), in_=sc_repl[kc, :, :]).then_inc(
                    s_wd[kc], 16
                )
                sync.dma_start(
                    out=xreg(0, kc), in_=xT[kc * P : (kc + 1) * P, 0:TCH]
                ).then_inc(s_x0[kc], 16)
            # x(1): lands well before tt=1 groups start
            for kc in range(KT):
                sync.dma_start(
                    out=xreg(1, kc), in_=xT[kc * P : (kc + 1) * P, TCH : 2 * TCH]
                ).then_inc(s_xc[0], 16)

            def ystore(n):
                tt, ot = n // OT, n % OT
                sync.wait_ge(s_act, n + 1)
                sync.dma_start(
                    out=yT[ot * P : (ot + 1) * P, tt * TCH : (tt + 1) * TCH],
                    in_=yslot(n),
                ).then_inc(s_ys[n % NYS], 16)

            for n in range(0, OT):
                ystore(n)
            # x(2) reuses buffer 0: all tt=0 groups must have read it
            sync.wait_ge(s_pe, OT * KT)
            for kc in range(KT):
                sync.dma_start(
                    out=xreg(2, kc), in_=xT[kc * P : (kc + 1) * P, 2 * TCH : 3 * TCH]
                ).then_inc(s_xc[1], 16)
            for n in range(OT, 2 * OT):
                ystore(n)
            # x(3) reuses buffer 1
            sync.wait_ge(s_pe, 2 * OT * KT)
            for kc in range(KT):
                sync.dma_start(
                    out=xreg(3, kc), in_=xT[kc * P : (kc + 1) * P, 3 * TCH : 4 * TCH]
                ).then_inc(s_xc[2], 16)
            for n in range(2 * OT, NG):
                ystore(n)
            for j in range(NYS):
                sync.wait_ge(s_ys[j], 16 * (NG // NYS))

        @block.vector
        def _(vector):
            vector.wait_ge(s_tbl, 32)
            nc.vector.reciprocal(out=inv_s[:, :], in_=s_sb[:, :]).then_inc(s_dq, 1)
            vector.wait_ge(s_dq, 1)  # recip retired before stt reads inv_s
            for kc in range(KT):
                vector.wait_ge(s_wd[kc], 32)  # w8 + scales tiles landed
                nc.vector.scalar_tensor_tensor(
                    wd(kc),
                    w8slot(kc),
                    inv_s[:, kc : kc + 1],
                    sslot(kc),
                    mybir.AluOpType.mult,
                    mybir.AluOpType.mult,
                ).then_inc(s_dq, 1)

        @block.scalar
        def _(scalar):
            scalar.wait_ge(s_tbl, 32)
            for n in range(NG):
                ot = n % OT
                if n >= NYS:
                    scalar.wait_ge(s_ys[n % NYS], 16 * (n // NYS))
                scalar.wait_ge(s_pe, _mm_end(n))
                nc.scalar.activation(
                    yslot(n),
                    ps[n % NB][:, :],
                    mybir.ActivationFunctionType.Identity,
                    bias=bias_sb[:, ot : ot + 1],
                    scale=1.0,
                ).then_inc(s_act, 1)

        @block.tensor
        def _(tensor):
            # wave 1: groups (tt=0, ot=0..7) accumulate kc-major
            for kc in range(KT):
                tensor.wait_ge(s_dq, kc + 2)  # dequant(kc) done (implies wd)
                tensor.wait_ge(s_x0[kc], 16)  # x(0) piece kc
                for b in range(WAVE):
                    nc.tensor.matmul(
                        ps[b][:, :],
                        wd(kc)[:, b * P : (b + 1) * P],
                        xreg(0, kc),
                        start=(kc == 0),
                        stop=(kc == KT - 1),
                    ).then_inc(s_pe, 1)
            # steady state: remaining groups, sequential
            for n in range(WAVE, NG):
                tt, ot = n // OT, n % OT
                if ot == 0 and tt > 0:
                    tensor.wait_ge(s_xc[tt - 1], 16 * KT)  # chunk tt loaded
                tensor.wait_ge(s_act, n - WAVE + 1)  # psum bank recycled
                for kc in range(KT):
                    nc.tensor.matmul(
                        ps[n % NB][:, :],
                        wd(kc)[:, ot * P : (ot + 1) * P],
                        xreg(tt, kc),
                        start=(kc == 0),
                        stop=(kc == KT - 1),
                    ).then_inc(s_pe, 1)

    return nc


def get_nc():
    if "nc" not in _NC_CACHE:
        _NC_CACHE["nc"] = _build_nc()
    return _NC_CACHE["nc"]


def _prep_inputs(x, w_q, scales, s, bias):
    import ml_dtypes

    bf16 = ml_dtypes.bfloat16
    fp8 = ml_dtypes.float8_e4m3
    x = np.asarray(x, dtype=np.float32)
    w_q = np.asarray(w_q)
    scales = np.asarray(scales, dtype=np.float32)
    s = np.asarray(s, dtype=np.float32)
    bias = np.asarray(bias, dtype=np.float32)

    pad = O_PAD - O_SHARD  # 32 rows of zero-padding per shard
    # weights: int in [-7,7] -> fp8e4m3 exact
    w = w_q.reshape(OUT, IN).astype(fp8)
    sc = scales.reshape(OUT, N_GROUPS)  # f32

    xT = np.ascontiguousarray(x.T.astype(bf16))  # [IN, TOKENS] bf16
    s_cols = np.ascontiguousarray(s.reshape(KT, P).T)  # [128, 32] f32

    in_maps = []
    for c in range(N_CORES):
        lo, hi = c * O_SHARD, (c + 1) * O_SHARD
        w_c = np.pad(w[lo:hi], ((0, pad), (0, 0)))  # [O_PAD, IN] fp8
        sc_c = np.pad(sc[lo:hi], ((0, pad), (0, 0)))  # [O_PAD, 32]
        b_c = np.pad(bias[lo:hi], (0, pad))  # [O_PAD]
        in_maps.append(
            {
                "xT": xT,
                "w8T": np.ascontiguousarray(w_c.T),  # [IN, O_PAD] fp8
                "sc_repl": np.ascontiguousarray(
                    np.broadcast_to(
                        sc_c.T.astype(bf16)[:, None, :], (N_GROUPS, P, O_PAD)
                    )
                ),  # [32, 128, O_PAD] bf16 (layout move only)
                "s_cols": s_cols,
                "bias_cols": np.ascontiguousarray(
                    b_c.reshape(OT, P).T
                ),  # [128, 11] f32
            }
        )
    return in_maps


def _install_profile_shim():
    """Provide antenv.axon_hooks (NTFF profiling via libaxon ctypes) when
    the container image lacks it. Only used for local perf iteration."""
    import contextlib
    import ctypes
    import sys
    import types

    if "antenv.axon_hooks" in sys.modules:
        return
    so_path = "/opt/axon/libaxon_pjrt.so"
    try:
        lib = ctypes.CDLL(so_path)
    except OSError:
        return
    if not hasattr(lib, "axon_start_nrt_profile"):
        return
    lib.axon_start_nrt_profile.argtypes = [
        ctypes.POINTER(ctypes.c_int64),
        ctypes.c_size_t,
    ]
    lib.axon_start_nrt_profile.restype = ctypes.c_int64
    lib.axon_stop_nrt_profile.argtypes = [ctypes.c_char_p]
    lib.axon_stop_nrt_profile.restype = ctypes.c_int64

    @contextlib.contextmanager
    def _hook(output_dir, device_ids):
        import jax

        jax.devices()
        if device_ids:
            ids = (ctypes.c_int64 * len(device_ids))(*device_ids)
            rc = lib.axon_start_nrt_profile(ids, len(device_ids))
        else:
            rc = lib.axon_start_nrt_profile(None, 0)
        if rc != 0:
            raise RuntimeError(f"axon_start_nrt_profile rc={rc}")
        try:
            yield
        finally:
            n = lib.axon_stop_nrt_profile(str(output_dir).encode())
            print(f"profile: {n} file(s) written to {output_dir}", file=sys.stderr)

    mod = types.ModuleType("antenv.axon_hooks")
    mod.get_axon_ntff_profile_hook = lambda: _hook
    mod.set_axon_ntff_profile_hook = lambda h: None
    sys.modules["antenv.axon_hooks"] = mod


def kernel(x, w_q, scales, s, bias):
    import sys

    if "/opt/trn_rl_repo" not in sys.path:
        sys.path.insert(0, "/opt/trn_rl_repo")
    import concourse.bass_utils as bass_utils
    from concourse.bass_utils import run_bass_kernel_spmd

    orig_dtype = np.asarray(x).dtype
    in_maps = _prep_inputs(x, w_q, scales, s, bias)
    nc = get_nc()

    trace = bool(os.environ.get("AWQ_TRACE"))
    kwargs = {}
    if trace:
        _install_profile_shim()
        bass_utils.upload_artifacts = lambda d: d  # zero-egress container
        tmpdir = os.environ.get("AWQ_TRACE_DIR")
        if tmpdir:
            os.makedirs(tmpdir, exist_ok=True)
            kwargs["tmpdir"] = tmpdir
    res = run_bass_kernel_spmd(
        nc,
        in_maps,
        core_ids=list(range(N_CORES)),
        trace=trace,
        **kwargs,
    )
    LAST["exec_time_ns"] = res.exec_time_ns
    LAST["results"] = res

    yT_full = np.concatenate(
        [np.asarray(res.results[c]["yT"], dtype=np.float32) for c in range(N_CORES)],
        axis=0,
    )  # [8*1408, 2048] f32
    y = np.ascontiguousarray(
        yT_full.reshape(N_CORES, O_PAD, TOKENS)[:, :O_SHARD, :]
        .reshape(OUT, TOKENS)
        .T
    )
    return y.astype(orig_dtype)
